# revision 14
# baseline (speedup 1.0000x reference)
"""LSTM cell with projection — Trainium2 Bass kernel.

Problem: B=64, T=512, I=512, H=512, C=4096 (4C=16384 gates), fp32 I/O.

Strategy (data-parallel over batch, per sharding hint):
  - 8 cores, 8 batch rows each; the recurrence is local per core (no collectives
    -- measured ~480us per AllReduce in this environment, which kills any
    tensor-parallel scheme at 512 sequential steps).
  - Transposed layout throughout: the gate dim lives on SBUF partitions
    (128 chunks x 128), batch lives in the free dim. All elementwise /
    activation work runs at full 128-lane width.
  - Matmuls run weight-stationary: bf16 weights -> LDWEIGHTS gets the
    compiler's fast-weight-load (2x), so W_state flows through the PE at
    ~the streaming rate while outputs come out already transposed.
  - Input projection (pin = x @ W_input.T + b) is computed per time-chunk
    inside the same NEFF, bounced through a DRAM scratch so its SBUF
    footprint stays small.
  - T=512 steps are split into CHUNK-step NEFF launches; h/c state rides
    DRAM between launches. Same NEFF re-launched with different inputs.
  - lengths are handled entirely host-side: dead rows keep computing
    garbage (rows are independent), and the host zeroes outputs past each
    row's length and extracts final h/c from the per-step dumps.

Gate reorder: rows of W_input/W_state/b_state are permuted to [i, f, o, m]
so sigmoid covers one contiguous slice and tanh another.
"""

import numpy as np
import ml_dtypes

import concourse.bass as bass
import concourse.bacc as bacc
import concourse.mybir as mybir
import concourse.tile as tile
from concourse.bass_utils import run_bass_kernel_spmd

B, T, I, H, C = 64, 512, 512, 512, 4096
G4 = 4 * C            # 16384 gate rows
NCORES = 8
BLOC = B // NCORES    # 8 batch rows per core
MC = G4 // 128        # 128 gate chunks
KH = H // 128         # 4 h-chunks (contraction for gates, M for proj)
KC = C // 128         # 32 cell chunks (contraction for proj)
MEM_CLIP = 3.0
PROJ_CLIP = 3.0

dtf = mybir.dt.float32
dtb = mybir.dt.bfloat16
AFT = mybir.ActivationFunctionType
ALU = mybir.AluOpType
bf16 = ml_dtypes.bfloat16

_NEFF_CACHE = {}


def _build(chunk: int):
    nc = bacc.Bacc(num_devices=NCORES)

    x_in = nc.dram_tensor("x", [BLOC, chunk, I], dtb, kind="ExternalInput")
    wst = [nc.dram_tensor(f"wst{k}", [128, G4], dtb, kind="ExternalInput")
           for k in range(KH)]
    wit = [nc.dram_tensor(f"wit{k}", [128, G4], dtb, kind="ExternalInput")
           for k in range(KH)]
    wpt = nc.dram_tensor("wpt", [128, KC * H], dtb, kind="ExternalInput")
    bt = nc.dram_tensor("bt", [128, MC], dtf, kind="ExternalInput")
    ht_in = nc.dram_tensor("ht_in", [128, KH * BLOC], dtb, kind="ExternalInput")
    c_in = nc.dram_tensor("c_in", [128, KC * BLOC], dtf, kind="ExternalInput")

    out_h = nc.dram_tensor("out_h", [chunk, 128, KH * BLOC], dtf, kind="ExternalOutput")
    out_c = nc.dram_tensor("out_c", [chunk, 128, KC * BLOC], dtf, kind="ExternalOutput")
    ht_out = nc.dram_tensor("ht_out", [128, KH * BLOC], dtb, kind="ExternalOutput")
    c_out = nc.dram_tensor("c_out", [128, KC * BLOC], dtf, kind="ExternalOutput")

    with tile.TileContext(nc) as tc:
        with (
            tc.tile_pool(name="wbig", bufs=1) as wbig,
            tc.tile_pool(name="state", bufs=1) as statep,
            tc.tile_pool(name="work", bufs=2) as workp,
            tc.tile_pool(name="wistg", bufs=3) as wistg,
            tc.tile_pool(name="pinw", bufs=3) as pinw,
            tc.tile_pool(name="psum", bufs=2, space="PSUM") as psump,
            tc.tile_pool(name="psg", bufs=1, space="PSUM") as psgp,
            tc.tile_pool(name="dram", bufs=1, space="DRAM") as dramp,
        ):
            # ---- persistent SBUF ----
            ws_sb = [wbig.tile([128, G4], dtb, tag=f"ws{k}", name=f"ws_sb{k}") for k in range(KH)]
            wp_sb = wbig.tile([128, KC * H], dtb, tag="wp")
            bt_sb = wbig.tile([128, MC], dtf, tag="bt")
            c_sb = statep.tile([128, KC * BLOC], dtf, tag="c")
            ht_sb = statep.tile([128, KH * BLOC], dtb, tag="ht")
            # f32 scratch: gsb | gact | t1 | t2 | th
            fw = statep.tile([128, 2 * MC * BLOC + 3 * KC * BLOC], dtf, tag="fw")
            NG = MC * BLOC          # 1024
            NB = KC * BLOC          # 256
            gsb = fw[:, 0:NG]
            gact = fw[:, NG:2 * NG]
            t1 = fw[:, 2 * NG:2 * NG + NB]
            t2 = fw[:, 2 * NG + NB:2 * NG + 2 * NB]
            th = fw[:, 2 * NG + 2 * NB:2 * NG + 3 * NB]
            sT = statep.tile([128, NB], dtb, tag="sT")
            osb = statep.tile([128, KH * BLOC], dtf, tag="osb")
            xT = statep.tile([128, KH, chunk, BLOC], dtb, tag="xT")

            ping = dramp.tile([MC, 128, chunk * BLOC], dtb)

            # ---- load weights / state ----
            for k in range(KH):
                for j in range(8):
                    sl = slice(j * (G4 // 8), (j + 1) * (G4 // 8))
                    nc.sync.dma_start(ws_sb[k][:, sl], wst[k][:, sl])
            for j in range(8):
                sl = slice(j * (KC * H // 8), (j + 1) * (KC * H // 8))
                nc.sync.dma_start(wp_sb[:, sl], wpt[:, sl])
            nc.sync.dma_start(bt_sb[:], bt[:])
            nc.sync.dma_start(c_sb[:], c_in[:])
            nc.sync.dma_start(ht_sb[:], ht_in[:])
            for k in range(KH):
                for b in range(BLOC):
                    nc.sync.dma_start(
                        xT[:, k, :, b],
                        x_in[b, :, k * 128:(k + 1) * 128].rearrange("t p -> p t"),
                    )

            # ---- phase A: pin.T for this chunk -> ping DRAM ----
            # pin.T[g, t, b] = sum_i Wi[g, i] * x[b, t, i]  + bias
            for mg in range(MC // 8):           # 16 groups of 8 m-tiles
                stg = [wistg.tile([128, 1024], dtb, tag=f"wis{k}", name=f"wis{k}_{mg}") for k in range(KH)]
                for k in range(KH):
                    nc.sync.dma_start(stg[k][:], wit[k][:, mg * 1024:(mg + 1) * 1024])
                for mm in range(8):
                    m = mg * 8 + mm
                    pp = psump.tile([128, chunk * BLOC], dtf, tag="pp")
                    for k in range(KH):
                        nc.tensor.matmul(
                            pp[:],
                            stg[k][:, mm * 128:(mm + 1) * 128],
                            xT[:, k, :, :],
                            start=(k == 0), stop=(k == KH - 1),
                        )
                    pst = pinw.tile([128, chunk * BLOC], dtb, tag="pst")
                    nc.scalar.activation(pst[:], pp[:], AFT.Identity,
                                         bias=bt_sb[:, m:m + 1])
                    nc.sync.dma_start(ping[m, :, :], pst[:])

            # ---- phase B: recurrence ----
            for t in range(chunk):
                pin_t = pinw.tile([128, MC, BLOC], dtb, tag="pin")
                nc.sync.dma_start(
                    pin_t[:],
                    ping[:, :, t * BLOC:(t + 1) * BLOC].rearrange("m p b -> p m b"),
                )

                # 4-way split: each gate group's add+activation overlaps the
                # next group's matmuls
                pgs = [psgp.tile([128, NB], dtf, tag=f"pg{q}", name=f"pg{q}_{t}")
                       for q in range(4)]
                pinf = pin_t[:, :, :].rearrange("p m b -> p (m b)")
                for q in range(4):
                    for mm in range(MC // 4):
                        m = q * (MC // 4) + mm
                        for k in range(KH):
                            nc.tensor.matmul(
                                pgs[q][:, mm * BLOC:(mm + 1) * BLOC],
                                ws_sb[k][:, m * 128:(m + 1) * 128],
                                ht_sb[:, k * BLOC:(k + 1) * BLOC],
                                start=(k == 0), stop=(k == KH - 1),
                            )
                    nc.vector.tensor_add(gsb[:, q * NB:(q + 1) * NB], pgs[q][:],
                                         pinf[:, q * NB:(q + 1) * NB])
                    nc.scalar.activation(
                        gact[:, q * NB:(q + 1) * NB], gsb[:, q * NB:(q + 1) * NB],
                        AFT.Tanh if q == 3 else AFT.Sigmoid)
                # mem = clip(i*m + f*c)
                nc.vector.tensor_mul(t1, gact[:, 0:NB], gact[:, 3 * NB:4 * NB])
                nc.vector.tensor_mul(t2, gact[:, NB:2 * NB], c_sb[:])
                nc.vector.tensor_add(t1, t1, t2)
                nc.vector.tensor_scalar(c_sb[:], t1, MEM_CLIP, -MEM_CLIP,
                                        ALU.min, ALU.max)
                nc.scalar.activation(th, c_sb[:], AFT.Tanh)
                nc.vector.tensor_mul(sT[:], gact[:, 2 * NB:3 * NB], th)

                ph = psgp.tile([128, KH * BLOC], dtf, tag="ph")
                for hm in range(KH):
                    for kc in range(KC):
                        nc.tensor.matmul(
                            ph[:, hm * BLOC:(hm + 1) * BLOC],
                            wp_sb[:, (kc * KH + hm) * 128:(kc * KH + hm + 1) * 128],
                            sT[:, kc * BLOC:(kc + 1) * BLOC],
                            start=(kc == 0), stop=(kc == KC - 1),
                        )
                nc.vector.tensor_scalar(osb[:], ph[:], PROJ_CLIP, -PROJ_CLIP,
                                        ALU.min, ALU.max)
                nc.vector.tensor_copy(ht_sb[:], osb[:])
                nc.sync.dma_start(out_h[t, :, :], osb[:])
                nc.sync.dma_start(out_c[t, :, :], c_sb[:])

            nc.sync.dma_start(ht_out[:], ht_sb[:])
            nc.sync.dma_start(c_out[:], c_sb[:])

    nc.finalize()
    return nc


def _prep_weights(W_input, W_state, b_state, W_proj):
    perm = np.concatenate([
        np.arange(0, C), np.arange(C, 2 * C),
        np.arange(3 * C, 4 * C), np.arange(2 * C, 3 * C),
    ])
    Wi = np.asarray(W_input)[perm]          # (16384, 512)
    Ws = np.asarray(W_state)[perm]
    br = np.asarray(b_state)[perm]
    Wp = np.asarray(W_proj)                 # (512, 4096)

    wst = [np.ascontiguousarray(Ws.T[k * 128:(k + 1) * 128]).astype(bf16)
           for k in range(KH)]
    wit = [np.ascontiguousarray(Wi.T[k * 128:(k + 1) * 128]).astype(bf16)
           for k in range(KH)]
    # wpt[p, kc*512 + hm*128 + col] = Wp[hm*128+col, kc*128+p]
    wpt = np.ascontiguousarray(
        Wp.T.reshape(KC, 128, KH, 128).transpose(1, 0, 2, 3).reshape(128, KC * H)
    ).astype(bf16)
    bt = np.ascontiguousarray(br.reshape(MC, 128).T).astype(np.float32)
    return wst, wit, wpt, bt


def _run(inputs, W_input, W_state, b_state, W_proj, t_total, chunk, lengths=None):
    key = chunk
    if key not in _NEFF_CACHE:
        _NEFF_CACHE[key] = _build(chunk)
    nc = _NEFF_CACHE[key]

    wst, wit, wpt, bt = _prep_weights(W_input, W_state, b_state, W_proj)
    x = np.asarray(inputs)

    ht = [np.zeros((128, KH * BLOC), dtype=bf16) for _ in range(NCORES)]
    cs = [np.zeros((128, KC * BLOC), dtype=np.float32) for _ in range(NCORES)]
    oh_all = [np.empty((t_total, 128, KH * BLOC), dtype=np.float32) for _ in range(NCORES)]
    oc_all = [np.empty((t_total, 128, KC * BLOC), dtype=np.float32) for _ in range(NCORES)]

    # per-core step budget from lengths (sorted desc -> core k's max len is
    # lengths[k*BLOC]); cores whose rows are all done drop out of later launches
    if lengths is not None:
        L = np.asarray(lengths).astype(np.int64)
        core_maxlen = [int(min(L[c * BLOC], t_total)) for c in range(NCORES)]
    else:
        core_maxlen = [t_total] * NCORES
    n_launch = -(-max(core_maxlen) // chunk)
    weights_maps = []
    for core in range(NCORES):
        m = {"wpt": wpt, "bt": bt}
        for k in range(KH):
            m[f"wst{k}"] = wst[k]
            m[f"wit{k}"] = wit[k]
        weights_maps.append(m)

    for j in range(n_launch):
        t0 = j * chunk
        n_act = max(1, sum(1 for c in range(NCORES) if core_maxlen[c] > t0))
        in_maps = []
        for core in range(n_act):
            xc = np.zeros((BLOC, chunk, I), dtype=bf16)
            xs = x[core * BLOC:(core + 1) * BLOC, t0:t0 + chunk, :]
            xc[:, :xs.shape[1], :] = xs.astype(bf16)
            im = dict(weights_maps[core])
            im["x"] = xc
            im["ht_in"] = ht[core]
            im["c_in"] = cs[core]
            in_maps.append(im)
        res = run_bass_kernel_spmd(nc, in_maps, core_ids=list(range(n_act)))
        tn = min(t0 + chunk, t_total)
        for core in range(n_act):
            r = res.results[core]
            oh_all[core][t0:tn] = r["out_h"][:tn - t0]
            oc_all[core][t0:tn] = r["out_c"][:tn - t0]
            ht[core] = r["ht_out"].astype(bf16)
            cs[core] = r["c_out"]
    return oh_all, oc_all


def _assemble(oh_all, oc_all, lengths, t_total):
    output = np.zeros((B, t_total, H), dtype=np.float32)
    final_h = np.zeros((1, B, H), dtype=np.float32)
    final_c = np.zeros((1, B, C), dtype=np.float32)
    L = np.asarray(lengths).astype(np.int64)
    for core in range(NCORES):
        # oh: (T, 128p, KH*BLOC) -> h[t, b, hm*128+p]
        oh = oh_all[core].reshape(t_total, 128, KH, BLOC)
        hval = oh.transpose(3, 0, 2, 1).reshape(BLOC, t_total, H)
        oc = oc_all[core].reshape(t_total, 128, KC, BLOC)
        for b in range(BLOC):
            row = core * BLOC + b
            ln = int(min(L[row], t_total))
            output[row, :ln] = hval[b, :ln]
            if ln > 0:
                final_h[0, row] = hval[b, ln - 1]
                final_c[0, row] = oc[ln - 1, :, :, b].T.reshape(C)
    return output, final_h, final_c


def kernel(inputs, lengths, W_input, W_state, b_state, W_proj, t_total=T, chunk=16):
    oh, oc = _run(inputs, W_input, W_state, b_state, W_proj, t_total, chunk,
                  lengths=lengths)
    return _assemble(oh, oc, lengths, t_total)


# revision 15
# speedup vs baseline: 1.4419x; 1.4419x over previous
"""LSTM cell with projection — Trainium2 Bass kernel.

Problem: B=64, T=512, I=512, H=512, C=4096 (4C=16384 gates), fp32 I/O.

Strategy (data-parallel over batch, per sharding hint):
  - 8 cores, 8 batch rows each; the recurrence is local per core (no collectives
    -- measured ~480us per AllReduce in this environment, which kills any
    tensor-parallel scheme at 512 sequential steps).
  - Transposed layout throughout: the gate dim lives on SBUF partitions
    (128 chunks x 128), batch lives in the free dim. All elementwise /
    activation work runs at full 128-lane width.
  - Matmuls run weight-stationary: bf16 weights -> LDWEIGHTS gets the
    compiler's fast-weight-load (2x), so W_state flows through the PE at
    ~the streaming rate while outputs come out already transposed.
  - Input projection (pin = x @ W_input.T + b) is computed per time-chunk
    inside the same NEFF, bounced through a DRAM scratch so its SBUF
    footprint stays small.
  - T=512 steps are split into CHUNK-step NEFF launches; h/c state rides
    DRAM between launches. Same NEFF re-launched with different inputs.
  - lengths are handled entirely host-side: dead rows keep computing
    garbage (rows are independent), and the host zeroes outputs past each
    row's length and extracts final h/c from the per-step dumps.

Gate reorder: rows of W_input/W_state/b_state are permuted to [i, f, o, m]
so sigmoid covers one contiguous slice and tanh another.
"""

import numpy as np
import ml_dtypes

import concourse.bass as bass
import concourse.bacc as bacc
import concourse.mybir as mybir
import concourse.tile as tile
from concourse.bass_utils import run_bass_kernel_spmd

B, T, I, H, C = 64, 512, 512, 512, 4096
G4 = 4 * C            # 16384 gate rows
NCORES = 8
BLOC = B // NCORES    # 8 batch rows per core
MC = G4 // 128        # 128 gate chunks
KH = H // 128         # 4 h-chunks (contraction for gates, M for proj)
KC = C // 128         # 32 cell chunks (contraction for proj)
MEM_CLIP = 3.0
PROJ_CLIP = 3.0

dtf = mybir.dt.float32
dtb = mybir.dt.bfloat16
AFT = mybir.ActivationFunctionType
ALU = mybir.AluOpType
bf16 = ml_dtypes.bfloat16

_NEFF_CACHE = {}


def _build(chunk: int):
    nc = bacc.Bacc(num_devices=NCORES)

    x_in = nc.dram_tensor("x", [BLOC, chunk, I], dtb, kind="ExternalInput")
    wst = [nc.dram_tensor(f"wst{k}", [128, G4], dtb, kind="ExternalInput")
           for k in range(KH)]
    wit = [nc.dram_tensor(f"wit{k}", [128, G4], dtb, kind="ExternalInput")
           for k in range(KH)]
    wpt = nc.dram_tensor("wpt", [128, KC * H], dtb, kind="ExternalInput")
    bt = nc.dram_tensor("bt", [128, MC], dtf, kind="ExternalInput")
    ht_in = nc.dram_tensor("ht_in", [128, KH * BLOC], dtb, kind="ExternalInput")
    c_in = nc.dram_tensor("c_in", [128, KC * BLOC], dtf, kind="ExternalInput")

    out_h = nc.dram_tensor("out_h", [chunk, 128, KH * BLOC], dtf, kind="ExternalOutput")
    out_c = nc.dram_tensor("out_c", [chunk, 128, KC * BLOC], dtf, kind="ExternalOutput")
    ht_out = nc.dram_tensor("ht_out", [128, KH * BLOC], dtb, kind="ExternalOutput")
    c_out = nc.dram_tensor("c_out", [128, KC * BLOC], dtf, kind="ExternalOutput")

    with tile.TileContext(nc) as tc:
        with (
            tc.tile_pool(name="wbig", bufs=1) as wbig,
            tc.tile_pool(name="state", bufs=1) as statep,
            tc.tile_pool(name="work", bufs=2) as workp,
            tc.tile_pool(name="wistg", bufs=3) as wistg,
            tc.tile_pool(name="pinw", bufs=3) as pinw,
            tc.tile_pool(name="psum", bufs=2, space="PSUM") as psump,
            tc.tile_pool(name="psg", bufs=1, space="PSUM") as psgp,
            tc.tile_pool(name="dram", bufs=1, space="DRAM") as dramp,
        ):
            # ---- persistent SBUF ----
            ws_sb = [wbig.tile([128, G4], dtb, tag=f"ws{k}", name=f"ws_sb{k}") for k in range(KH)]
            wp_sb = wbig.tile([128, KC * H], dtb, tag="wp")
            bt_sb = wbig.tile([128, MC], dtf, tag="bt")
            c_sb = statep.tile([128, KC * BLOC], dtf, tag="c")
            ht_sb = statep.tile([128, KH * BLOC], dtb, tag="ht")
            # f32 scratch: gsb | gact | t1 | t2 | th
            fw = statep.tile([128, 2 * MC * BLOC + 3 * KC * BLOC], dtf, tag="fw")
            NG = MC * BLOC          # 1024
            NB = KC * BLOC          # 256
            gsb = fw[:, 0:NG]
            gact = fw[:, NG:2 * NG]
            t1 = fw[:, 2 * NG:2 * NG + NB]
            t2 = fw[:, 2 * NG + NB:2 * NG + 2 * NB]
            th = fw[:, 2 * NG + 2 * NB:2 * NG + 3 * NB]
            sT = statep.tile([128, NB], dtb, tag="sT")
            osb = statep.tile([128, KH * BLOC], dtf, tag="osb")
            xT = statep.tile([128, KH, chunk, BLOC], dtb, tag="xT")

            ping = dramp.tile([MC, 128, chunk * BLOC], dtb)

            # ---- load weights / state ----
            for k in range(KH):
                for j in range(8):
                    sl = slice(j * (G4 // 8), (j + 1) * (G4 // 8))
                    nc.sync.dma_start(ws_sb[k][:, sl], wst[k][:, sl])
            for j in range(8):
                sl = slice(j * (KC * H // 8), (j + 1) * (KC * H // 8))
                nc.sync.dma_start(wp_sb[:, sl], wpt[:, sl])
            nc.sync.dma_start(bt_sb[:], bt[:])
            nc.sync.dma_start(c_sb[:], c_in[:])
            nc.sync.dma_start(ht_sb[:], ht_in[:])
            for k in range(KH):
                for b in range(BLOC):
                    nc.sync.dma_start(
                        xT[:, k, :, b],
                        x_in[b, :, k * 128:(k + 1) * 128].rearrange("t p -> p t"),
                    )

            # ---- phase A: pin.T for this chunk -> ping DRAM ----
            # pin.T[g, t, b] = sum_i Wi[g, i] * x[b, t, i]  + bias
            for mg in range(MC // 8):           # 16 groups of 8 m-tiles
                stg = [wistg.tile([128, 1024], dtb, tag=f"wis{k}", name=f"wis{k}_{mg}") for k in range(KH)]
                for k in range(KH):
                    nc.sync.dma_start(stg[k][:], wit[k][:, mg * 1024:(mg + 1) * 1024])
                for mm in range(8):
                    m = mg * 8 + mm
                    pp = psump.tile([128, chunk * BLOC], dtf, tag="pp")
                    for k in range(KH):
                        nc.tensor.matmul(
                            pp[:],
                            stg[k][:, mm * 128:(mm + 1) * 128],
                            xT[:, k, :, :],
                            start=(k == 0), stop=(k == KH - 1),
                        )
                    pst = pinw.tile([128, chunk * BLOC], dtb, tag="pst")
                    nc.scalar.activation(pst[:], pp[:], AFT.Identity,
                                         bias=bt_sb[:, m:m + 1])
                    nc.sync.dma_start(ping[m, :, :], pst[:])

            # ---- phase B: recurrence ----
            for t in range(chunk):
                pin_t = pinw.tile([128, MC, BLOC], dtb, tag="pin")
                nc.sync.dma_start(
                    pin_t[:],
                    ping[:, :, t * BLOC:(t + 1) * BLOC].rearrange("m p b -> p m b"),
                )

                # 4-way split: each gate group's add+activation overlaps the
                # next group's matmuls
                pgs = [psgp.tile([128, NB], dtf, tag=f"pg{q}", name=f"pg{q}_{t}")
                       for q in range(4)]
                pinf = pin_t[:, :, :].rearrange("p m b -> p (m b)")
                for q in range(4):
                    for mm in range(MC // 4):
                        m = q * (MC // 4) + mm
                        for k in range(KH):
                            nc.tensor.matmul(
                                pgs[q][:, mm * BLOC:(mm + 1) * BLOC],
                                ws_sb[k][:, m * 128:(m + 1) * 128],
                                ht_sb[:, k * BLOC:(k + 1) * BLOC],
                                start=(k == 0), stop=(k == KH - 1),
                            )
                    nc.vector.tensor_add(gsb[:, q * NB:(q + 1) * NB], pgs[q][:],
                                         pinf[:, q * NB:(q + 1) * NB])
                    nc.scalar.activation(
                        gact[:, q * NB:(q + 1) * NB], gsb[:, q * NB:(q + 1) * NB],
                        AFT.Tanh if q == 3 else AFT.Sigmoid)
                # mem = clip(i*m + f*c)
                nc.vector.tensor_mul(t1, gact[:, 0:NB], gact[:, 3 * NB:4 * NB])
                nc.vector.tensor_mul(t2, gact[:, NB:2 * NB], c_sb[:])
                nc.vector.tensor_add(t1, t1, t2)
                nc.vector.tensor_scalar(c_sb[:], t1, MEM_CLIP, -MEM_CLIP,
                                        ALU.min, ALU.max)
                nc.scalar.activation(th, c_sb[:], AFT.Tanh)
                nc.vector.tensor_mul(sT[:], gact[:, 2 * NB:3 * NB], th)

                ph = psgp.tile([128, KH * BLOC], dtf, tag="ph")
                for hm in range(KH):
                    for kc in range(KC):
                        nc.tensor.matmul(
                            ph[:, hm * BLOC:(hm + 1) * BLOC],
                            wp_sb[:, (kc * KH + hm) * 128:(kc * KH + hm + 1) * 128],
                            sT[:, kc * BLOC:(kc + 1) * BLOC],
                            start=(kc == 0), stop=(kc == KC - 1),
                        )
                nc.vector.tensor_scalar(osb[:], ph[:], PROJ_CLIP, -PROJ_CLIP,
                                        ALU.min, ALU.max)
                nc.vector.tensor_copy(ht_sb[:], osb[:])
                nc.sync.dma_start(out_h[t, :, :], osb[:])
                nc.sync.dma_start(out_c[t, :, :], c_sb[:])

            nc.sync.dma_start(ht_out[:], ht_sb[:])
            nc.sync.dma_start(c_out[:], c_sb[:])

    nc.finalize()
    return nc


def _prep_weights(W_input, W_state, b_state, W_proj):
    perm = np.concatenate([
        np.arange(0, C), np.arange(C, 2 * C),
        np.arange(3 * C, 4 * C), np.arange(2 * C, 3 * C),
    ])
    Wi = np.asarray(W_input)[perm]          # (16384, 512)
    Ws = np.asarray(W_state)[perm]
    br = np.asarray(b_state)[perm]
    Wp = np.asarray(W_proj)                 # (512, 4096)

    wst = [np.ascontiguousarray(Ws.T[k * 128:(k + 1) * 128]).astype(bf16)
           for k in range(KH)]
    wit = [np.ascontiguousarray(Wi.T[k * 128:(k + 1) * 128]).astype(bf16)
           for k in range(KH)]
    # wpt[p, kc*512 + hm*128 + col] = Wp[hm*128+col, kc*128+p]
    wpt = np.ascontiguousarray(
        Wp.T.reshape(KC, 128, KH, 128).transpose(1, 0, 2, 3).reshape(128, KC * H)
    ).astype(bf16)
    bt = np.ascontiguousarray(br.reshape(MC, 128).T).astype(np.float32)
    return wst, wit, wpt, bt


def _run(inputs, W_input, W_state, b_state, W_proj, t_total, chunk, lengths=None):
    key = chunk
    if key not in _NEFF_CACHE:
        _NEFF_CACHE[key] = _build(chunk)
    nc = _NEFF_CACHE[key]

    wst, wit, wpt, bt = _prep_weights(W_input, W_state, b_state, W_proj)
    x = np.asarray(inputs)

    ht = [np.zeros((128, KH * BLOC), dtype=bf16) for _ in range(NCORES)]
    cs = [np.zeros((128, KC * BLOC), dtype=np.float32) for _ in range(NCORES)]
    oh_all = [np.empty((t_total, 128, KH * BLOC), dtype=np.float32) for _ in range(NCORES)]
    oc_all = [np.empty((t_total, 128, KC * BLOC), dtype=np.float32) for _ in range(NCORES)]

    # per-core step budget from lengths (sorted desc -> core k's max len is
    # lengths[k*BLOC]); cores whose rows are all done drop out of later launches
    if lengths is not None:
        L = np.asarray(lengths).astype(np.int64)
        core_maxlen = [int(min(L[c * BLOC], t_total)) for c in range(NCORES)]
    else:
        core_maxlen = [t_total] * NCORES
    n_launch = -(-max(core_maxlen) // chunk)
    weights_maps = []
    for core in range(NCORES):
        m = {"wpt": wpt, "bt": bt}
        for k in range(KH):
            m[f"wst{k}"] = wst[k]
            m[f"wit{k}"] = wit[k]
        weights_maps.append(m)

    for j in range(n_launch):
        t0 = j * chunk
        n_act = max(1, sum(1 for c in range(NCORES) if core_maxlen[c] > t0))
        in_maps = []
        for core in range(n_act):
            xc = np.zeros((BLOC, chunk, I), dtype=bf16)
            xs = x[core * BLOC:(core + 1) * BLOC, t0:t0 + chunk, :]
            xc[:, :xs.shape[1], :] = xs.astype(bf16)
            im = dict(weights_maps[core])
            im["x"] = xc
            im["ht_in"] = ht[core]
            im["c_in"] = cs[core]
            in_maps.append(im)
        res = run_bass_kernel_spmd(nc, in_maps, core_ids=list(range(n_act)))
        tn = min(t0 + chunk, t_total)
        for core in range(n_act):
            r = res.results[core]
            oh_all[core][t0:tn] = r["out_h"][:tn - t0]
            oc_all[core][t0:tn] = r["out_c"][:tn - t0]
            ht[core] = r["ht_out"].astype(bf16)
            cs[core] = r["c_out"]
    return oh_all, oc_all


def _assemble(oh_all, oc_all, lengths, t_total):
    output = np.zeros((B, t_total, H), dtype=np.float32)
    final_h = np.zeros((1, B, H), dtype=np.float32)
    final_c = np.zeros((1, B, C), dtype=np.float32)
    L = np.asarray(lengths).astype(np.int64)
    for core in range(NCORES):
        # oh: (T, 128p, KH*BLOC) -> h[t, b, hm*128+p]
        oh = oh_all[core].reshape(t_total, 128, KH, BLOC)
        hval = oh.transpose(3, 0, 2, 1).reshape(BLOC, t_total, H)
        oc = oc_all[core].reshape(t_total, 128, KC, BLOC)
        for b in range(BLOC):
            row = core * BLOC + b
            ln = int(min(L[row], t_total))
            output[row, :ln] = hval[b, :ln]
            if ln > 0:
                final_h[0, row] = hval[b, ln - 1]
                final_c[0, row] = oc[ln - 1, :, :, b].T.reshape(C)
    return output, final_h, final_c


def kernel(inputs, lengths, W_input, W_state, b_state, W_proj, t_total=T, chunk=32):
    oh, oc = _run(inputs, W_input, W_state, b_state, W_proj, t_total, chunk,
                  lengths=lengths)
    return _assemble(oh, oc, lengths, t_total)
